# revision 20
# baseline (speedup 1.0000x reference)
"""Trainium2 Bass kernel for nn_CausalSelfAttention (B=1, T=2048, D=1024, H=16).

Sharding: 2 heads per core across 8 cores (tensor parallel). Wq/Wk/Wv
column-sharded by head, attention fully local, Wo row-sharded; host sums the
8 partial outputs (the all-reduce of the unshard step).

All-bf16 pipeline (PSUM accumulation stays f32):
  P1  fused QKV token-major: psum[t,384] = sum_i xT_blk.T @ [WqT|WkT|(1-l)WvT]
      (bf16 stationary -> FWL). No evacuation: Square (ACT), rope swap (ACT),
      cos-mult (DVE), v-blend (DVE) read the PSUM tile directly; sin-mult
      on Pool from SBUF. RMS stats -> bit-trick rsqrt (DVE). q normalized
      token-major; k's scale (x0.125) folds into the Exp scale operand.
  P2  DMA-XBAR transposes (2-byte) build qT/kT directly in SBUF - no PE
      transposes, no PSUM, no evacuation copies.
  P3  attention per (pair p of 1024 queries, head h), phase-split: phase A
      emits all score matmuls (<=512 moving) + span-trimmed Exp with
      scale=rk (ACT) + tri-mask (DVE), keeping every E tile in SBUF;
      phase B streams the [v|1|0pad].T @ E accumulation matmuls
      back-to-back (wait-free -> PE stays busy -> HAM keeps full clock).
      Windows interleave B(prev) with A(next) and the QKV tail.
  P4  scale = 1/(L + e^sink) broadcast via K=2 matmul + reciprocal ->
      yts bf16; out-proj per 128-col Wo chunk (bf16, FWL), f32 partials.
"""

import sys

if "/opt/trn_rl_repo" not in sys.path:
    sys.path.insert(0, "/opt/trn_rl_repo")

import numpy as np
import ml_dtypes
from contextlib import ExitStack

from concourse import bacc, tile
from concourse import mybir
from concourse.bass_utils import run_bass_kernel_spmd

F32 = mybir.dt.float32
F32R = mybir.dt.float32r
BF16 = mybir.dt.bfloat16
I32 = mybir.dt.int32
AF = mybir.ActivationFunctionType
ALU = mybir.AluOpType
AX = mybir.AxisListType

T = 2048
D = 1024
HD = 64
NT = T // 128  # 16 t-tiles
RMS_EPS = float(np.finfo(np.float32).eps)


def _build_program():
    nc = bacc.Bacc("TRN2", target_bir_lowering=False, debug=False, num_devices=8)

    d_xtb = nc.dram_tensor("xtb", [NT, 128, 8, 128], BF16, kind="ExternalInput").ap()
    d_wqkv = nc.dram_tensor("wqkv", [128, 8, 384], BF16, kind="ExternalInput").ap()
    d_vis = nc.dram_tensor("vis", [128, NT, 128], BF16, kind="ExternalInput").ap()
    d_cc = nc.dram_tensor("cc", [128, NT, 64], BF16, kind="ExternalInput").ap()
    d_sc = nc.dram_tensor("sc", [128, NT, 64], BF16, kind="ExternalInput").ap()
    d_wo = nc.dram_tensor("wo", [128, D], BF16, kind="ExternalInput").ap()
    d_idn = nc.dram_tensor("idn", [128, 128], BF16, kind="ExternalInput").ap()
    d_tri = nc.dram_tensor("tri", [128, 128], BF16, kind="ExternalInput").ap()
    d_onp = nc.dram_tensor("onp", [66, 128], F32R, kind="ExternalInput").ap()
    d_lsbi = nc.dram_tensor("lsbi", [1, 4096], F32R, kind="ExternalInput").ap()
    d_out = nc.dram_tensor("out", [D, T], F32, kind="ExternalOutput").ap()

    with tile.TileContext(nc) as tc, ExitStack() as ctx:
        sb = ctx.enter_context(tc.tile_pool(name="sb", bufs=1))
        sb_x = ctx.enter_context(tc.tile_pool(name="sb_x", bufs=6))
        sb_w1 = ctx.enter_context(tc.tile_pool(name="sb_w1", bufs=3))
        sb_w2 = ctx.enter_context(tc.tile_pool(name="sb_w2", bufs=5))
        sb_e = ctx.enter_context(tc.tile_pool(name="sb_e", bufs=30))
        sb_o = ctx.enter_context(tc.tile_pool(name="sb_o", bufs=3))
        ps = ctx.enter_context(tc.tile_pool(name="ps", bufs=2, space="PSUM"))
        ps_st = ctx.enter_context(tc.tile_pool(name="ps_st", bufs=2, space="PSUM"))
        ps_yt = ctx.enter_context(tc.tile_pool(name="ps_yt", bufs=1, space="PSUM"))

        # weights first on the sync queue (needed by the first matmul);
        # other constants go via the gpsimd queue so they don't delay x.
        wqkv = sb.tile([128, 8, 384], BF16)
        for _wp in range(3):
            nc.sync.dma_start(
                out=wqkv[:, :, 128 * _wp : 128 * (_wp + 1)],
                in_=d_wqkv[:, :, 128 * _wp : 128 * (_wp + 1)],
            )
        vi_t = sb.tile([128, NT, 128], BF16)
        cc_t = sb.tile([128, NT, 64], BF16)
        sc_t = sb.tile([128, NT, 64], BF16)
        wo = sb.tile([128, D], BF16)
        idn = sb.tile([128, 128], BF16)
        tri = sb.tile([128, 128], BF16)
        onp = sb.tile([66, 128], F32R)
        lsb = sb.tile([66, 4096], F32R)
        const_dmas = []
        early_dmas = []
        early_dmas.append(nc.gpsimd.dma_start(out=vi_t[:], in_=d_vis[:]))
        early_dmas.append(nc.gpsimd.dma_start(out=cc_t[:], in_=d_cc[:]))
        early_dmas.append(nc.gpsimd.dma_start(out=sc_t[:], in_=d_sc[:]))
        early_dmas.append(nc.gpsimd.dma_start(out=idn[:], in_=d_idn[:]))
        const_dmas.append(nc.gpsimd.dma_start(out=wo[:], in_=d_wo[:]))
        const_dmas.append(nc.gpsimd.dma_start(out=tri[:], in_=d_tri[:]))
        const_dmas.append(nc.gpsimd.dma_start(out=onp[:], in_=d_onp[:]))
        const_dmas.append(nc.gpsimd.dma_start(out=lsb[65:66, :], in_=d_lsbi[:]))

        stats = sb.tile([128, 64], F32)
        rbuf = sb.tile([128, 64], F32)
        qT = sb.tile([128, T], BF16)
        kT = sb.tile([128, T], BF16)
        # [v_h | 1 | 0*63] per head -> 128-col stationary (FWL + L row)
        vtiles = [sb.tile([128, 2, 128], BF16, tag=f"v{i}", name=f"v{i}") for i in range(NT)]
        qkro = [sb.tile([128, 256], BF16, tag=f"qkro{i}", name=f"qkro{i}") for i in range(NT)]
        yts = sb.tile([128, T], BF16)

        # ---------------- emission helpers ----------------
        from concourse.tile import add_dep_helper

        first_mm = [None]
        xt0_dma = [None]

        def emit_qkv_tile(ti):
            xt = sb_x.tile([128, 8, 128], BF16, tag="xt", name=f"xt{ti}")
            nc.sync.dma_start(out=xt[:, 0:4, :], in_=d_xtb[ti, :, 0:4, :])
            dma = nc.sync.dma_start(out=xt[:, 4:8, :], in_=d_xtb[ti, :, 4:8, :])
            if ti == 0:
                xt0_dma[0] = dma
            psq = ps.tile([128, 384], F32, tag="qkv", name=f"psq{ti}")
            for i in range(8):
                mm = nc.tensor.matmul(
                    psq[:], xt[:, i, :], wqkv[:, i, :],
                    start=(i == 0), stop=(i == 7),
                )
            if ti == 0:
                first_mm[0] = mm
                for cd in const_dmas:
                    add_dep_helper(cd.ins, mm.ins, True, "defer const DMA")
                for cd in early_dmas:
                    add_dep_helper(cd.ins, xt0_dma[0].ins, True, "defer early DMA")
            # stats: Square (ACT, bf16 out) + segmented reduce (DVE)
            sqt = sb_w1.tile([128, 256], BF16, tag="sqt", name=f"sqt{ti}")
            nc.scalar.activation(sqt[:], psq[:, 0:256], AF.Square)
            nc.vector.tensor_reduce(
                stats[:, 4 * ti : 4 * ti + 4],
                sqt[:].rearrange("p (s c) -> p s c", s=4),
                axis=AX.X, op=ALU.add,
            )
            # v-blend on DVE (Pool cannot read PSUM)
            vt = vtiles[ti]
            nc.vector.tensor_tensor(
                out=vt[:, :, 0:64],
                in0=psq[:, 256:384].rearrange("p (s c) -> p s c", s=2),
                in1=vi_t[:, ti, :].rearrange("p (s c) -> p s c", s=2),
                op=ALU.add,
            )
            # rope: cos-mult (DVE from psum), sin-mult via swapped-half
            # slices (2 DVE ops, no ACT copies) -> add (DVE, all-sbuf bf16)
            tcos = sb_w1.tile([128, 256], BF16, tag="tcos", name=f"tcos{ti}")
            nc.vector.tensor_tensor(
                out=tcos[:].rearrange("p (s c) -> p s c", s=4),
                in0=psq[:, 0:256].rearrange("p (s c) -> p s c", s=4),
                in1=cc_t[:, ti, :].unsqueeze(1).broadcast_to((128, 4, 64)),
                op=ALU.mult,
            )
            tsin = sb_w1.tile([128, 256], BF16, tag="tsin", name=f"tsin{ti}")
            q4 = psq[:, 0:256].rearrange("p (s h c) -> p s h c", s=4, h=2)
            t4 = tsin[:].rearrange("p (s h c) -> p s h c", s=4, h=2)
            for hh in range(2):
                nc.vector.tensor_tensor(
                    out=t4[:, :, hh, :],
                    in0=q4[:, :, 1 - hh, :],
                    in1=sc_t[:, ti, 32 * hh : 32 * hh + 32]
                    .unsqueeze(1)
                    .broadcast_to((128, 4, 32)),
                    op=ALU.mult,
                )
            nc.gpsimd.tensor_tensor(
                out=qkro[ti][:], in0=tcos[:], in1=tsin[:], op=ALU.add
            )

        def emit_chain(g):
            # batched rsqrt for tiles 4g..4g+3 (DVE bit-trick + 2 Newton
            # iters); k cols get 0.125 folded (consumed by the Exp scale)
            gg = 16 * g
            rs = rbuf[:, gg : gg + 16]
            zt = sb_w2.tile([128, 16], F32, tag="zt", name=f"zt{g}")
            nt1 = sb_w2.tile([128, 16], F32, tag="nt1", name=f"nt1{g}")
            nc.vector.tensor_scalar(
                out=zt[:], in0=stats[:, gg : gg + 16], scalar1=1.0 / 64.0,
                scalar2=RMS_EPS, op0=ALU.mult, op1=ALU.add,
            )
            nc.vector.tensor_scalar(
                out=nt1[:].bitcast(I32), in0=zt[:].bitcast(I32), scalar1=1,
                scalar2=0xFFFFFFFF, op0=ALU.logical_shift_right,
                op1=ALU.bitwise_xor,
            )
            nc.vector.tensor_scalar(
                out=rs.bitcast(I32), in0=nt1[:].bitcast(I32),
                scalar1=0x5F3759E0, scalar2=None, op0=ALU.add,
            )
            for _ in range(2):
                nc.vector.tensor_tensor(out=nt1[:], in0=rs, in1=rs, op=ALU.mult)
                nc.vector.tensor_tensor(out=nt1[:], in0=nt1[:], in1=zt[:], op=ALU.mult)
                nc.vector.tensor_scalar(
                    out=nt1[:], in0=nt1[:], scalar1=-0.5, scalar2=1.5,
                    op0=ALU.mult, op1=ALU.add,
                )
                nc.vector.tensor_tensor(out=rs, in0=rs, in1=nt1[:], op=ALU.mult)
            kv = rbuf[:, gg : gg + 16].rearrange("p (t c) -> p t c", c=4)[:, :, 2:4]
            nc.vector.tensor_scalar_mul(kv, kv, 0.125)

        def emit_trq(tj):
            # qnorm (Pool) then PE transpose into qT (DVE evac)
            qn = sb_w2.tile([128, 128], BF16, tag="qkrq", name=f"qkrq{tj}")
            nc.gpsimd.tensor_tensor(
                out=qn[:].rearrange("p (s c) -> p s c", s=2),
                in0=qkro[tj][:, 0:128].rearrange("p (s c) -> p s c", s=2),
                in1=rbuf[:, 4 * tj : 4 * tj + 2]
                .unsqueeze(2)
                .broadcast_to((128, 2, 64)),
                op=ALU.mult,
            )
            ptr = ps.tile([128, 128], BF16, tag="qkv", name=f"trq{tj}")
            nc.tensor.transpose(ptr[:], qn[:], idn[:])
            nc.vector.tensor_copy(qT[:, 128 * tj : 128 * (tj + 1)], ptr[:])

        def emit_trk(tj):
            ptr2 = ps.tile([128, 128], BF16, tag="qkv", name=f"trk{tj}")
            nc.tensor.transpose(ptr2[:], qkro[tj][:, 128:256], idn[:])
            nc.vector.tensor_copy(kT[:, 128 * tj : 128 * (tj + 1)], ptr2[:])

        def spans(qs):
            # moving free dim max is 512: split [qs,1024) at 512
            return [(qs, 512), (512, 1024)] if qs < 512 else [(qs, 1024)]

        ets = {}

        def s_unit(p, h, kj):
            # score block + Exp(scale=rk) + diagonal tri-mask -> et kept
            qs = max(0, 128 * kj - 1024 * p)
            st = ps_st.tile([128, 1024], F32, tag="st", name=f"st{p}_{h}_{kj}")
            for a, bnd in spans(qs):
                nc.tensor.matmul(
                    st[:, a:bnd],
                    kT[64 * h : 64 * h + 64, 128 * kj : 128 * (kj + 1)],
                    qT[64 * h : 64 * h + 64, 1024 * p + a : 1024 * p + bnd],
                    start=True, stop=True,
                )
            et = sb_e.tile([128, 1024], BF16, tag="et", name=f"et{p}_{h}_{kj}")
            nc.scalar.activation(
                et[:, qs:1024], st[:, qs:1024], AF.Exp,
                scale=rbuf[:, 4 * kj + 2 + h : 4 * kj + 3 + h],
            )
            if kj >= 8 * p:  # diagonal block
                blk = et[:, qs : qs + 128]
                nc.vector.tensor_tensor(out=blk, in0=blk, in1=tri[:], op=ALU.mult)
            ets[(p, h, kj)] = et

        yt_tiles = {}

        def y_unit(p, h, kj):
            kjmax = 8 * p + 8
            if kj == 0:
                yt_tiles[(p, h)] = ps_yt.tile(
                    [128, 1024], F32, tag="yt", name=f"yt{p}_{h}"
                )
            yt = yt_tiles[(p, h)]
            qs = max(0, 128 * kj - 1024 * p)
            et = ets.pop((p, h, kj))
            for a, bnd in spans(qs):
                nc.tensor.matmul(
                    yt[:, a:bnd], vtiles[kj][:, h, :], et[:, a:bnd],
                    start=(kj == 0), stop=(kj == kjmax - 1 and bnd == 1024),
                )

        ytr_tiles = {}

        def scale1(p, h):
            yt = yt_tiles.pop((p, h))
            ytr = sb_o.tile([65, 1024], F32, tag="ytr", name=f"ytr{p}_{h}", bufs=2)
            if p == 0:
                nc.vector.tensor_copy(ytr[:], yt[0:65, 0:1024])
            else:
                nc.scalar.copy(ytr[:], yt[0:65, 0:1024])
            ytr_tiles[(p, h)] = ytr

        def scale2(p, h):
            ytr = ytr_tiles.pop((p, h))
            slot = (2 * p + h) * 1024
            nc.gpsimd.tensor_copy(
                out=lsb[64:65, slot : slot + 1024], in_=ytr[64:65, :]
            )
            mbs = sb_o.tile([64, 1024], F32, tag="mbs", name=f"mbs{p}_{h}")
            for half in range(2):
                mb = ps.tile([64, 512], F32, tag="qkv", name=f"mb{p}_{h}_{half}")
                nc.tensor.matmul(
                    mb[0:64, :],
                    onp[64:66, 64 * h : 64 * h + 64],
                    lsb[64:66, slot + 512 * half : slot + 512 * (half + 1)],
                    start=True, stop=True,
                )
                nc.vector.reciprocal_approx_fast(
                    out=mbs[:, 512 * half : 512 * (half + 1)], in_=mb[0:64, :]
                )
            if h == 0:
                nc.vector.tensor_tensor(
                    out=yts[0:64, 1024 * p : 1024 * (p + 1)],
                    in0=ytr[0:64, :], in1=mbs[:], op=ALU.mult,
                )
            else:
                yts1 = sb_o.tile([64, 1024], BF16, tag="yts1", name=f"yts1_{p}")
                nc.vector.tensor_tensor(
                    out=yts1[:], in0=ytr[0:64, :], in1=mbs[:], op=ALU.mult
                )
                nc.gpsimd.dma_start(
                    out=yts[64:128, 1024 * p : 1024 * (p + 1)], in_=yts1[:]
                )

        def op_unit(p, jt, use_act=False):
            for half in range(2):
                pso = ps.tile([128, 512], F32, tag="qkv", name=f"pso{p}_{jt}_{half}")
                lo = 1024 * p + 512 * half
                nc.tensor.matmul(
                    pso[:],
                    wo[:, 128 * jt : 128 * (jt + 1)],
                    yts[:, lo : lo + 512],
                    start=True, stop=True,
                )
                outsb = sb_o.tile(
                    [128, 512], F32, tag="outsb", name=f"osb{p}_{jt}_{half}"
                )
                if use_act and (jt + half) % 2 == 0:
                    nc.scalar.copy(outsb[:], pso[:])
                else:
                    nc.vector.tensor_copy(outsb[:], pso[:])
                q = nc.sync if jt % 2 == 0 else nc.gpsimd
                q.dma_start(
                    out=d_out[128 * jt : 128 * (jt + 1), lo : lo + 512],
                    in_=outsb[:],
                )

        # ---------------- HAM warm-up + one-time memsets ----------------
        wz = sb.tile([128, 512], BF16)
        nc.gpsimd.memset(wz[:], 0.0)
        pwz = ps_st.tile([128, 1024], F32, tag="st", name="pwz")
        for _w in range(10):
            nc.tensor.matmul(pwz[:, 0:512], wz[:, 0:128], wz[:], start=True, stop=True)
        for ti in range(NT):
            nc.gpsimd.memset(vtiles[ti][:, :, 64:65], 1.0)
            nc.gpsimd.memset(vtiles[ti][:, :, 65:128], 0.0)

        # ---------------- windowed emission ----------------
        # head: QKV 0-7; k-transposes eagerly (no chain dep), q-transposes
        # right after each Pool rsqrt chain
        for ti in range(4):
            emit_qkv_tile(ti)
        emit_chain(0)
        emit_trk(0)
        emit_trk(1)
        emit_qkv_tile(4)
        emit_trq(0)
        emit_trk(2)
        emit_qkv_tile(5)
        emit_trq(1)
        emit_trk(3)
        emit_qkv_tile(6)
        emit_trq(2)
        emit_trq(3)
        emit_qkv_tile(7)
        emit_chain(1)
        for tj in range(4, 8):
            emit_trk(tj)
            emit_trq(tj)

        # w1: A(p0,h0) + A(p0,h1) interleaved with QKV tiles 8-15
        for kj in range(8):
            s_unit(0, 0, kj)
            emit_qkv_tile(8 + kj)
            s_unit(0, 1, kj)
            if kj >= 2:
                emit_trk(6 + kj)
        emit_chain(2)
        emit_trk(14)
        emit_trk(15)
        for tj in range(8, 12):
            emit_trq(tj)
        emit_chain(3)
        for tj in range(12, 16):
            emit_trq(tj)

        # w2: B(p0,h0) streaming, A(p1,h0) kj 0-7 (full-span scores)
        for j in range(4):
            y_unit(0, 0, j)
        for j in range(4):
            y_unit(0, 0, 4 + j)
            s_unit(1, 0, j)
        for j in range(4, 8):
            s_unit(1, 0, j)
        scale1(0, 0)

        # w3: B(p0,h1) streaming, A(p1,h0) kj 8-15
        scale2(0, 0)
        s_unit(1, 0, 8)
        s_unit(1, 0, 9)
        for j in range(6):
            y_unit(0, 1, j)
            s_unit(1, 0, 10 + j)
        y_unit(0, 1, 6)
        y_unit(0, 1, 7)
        scale1(0, 1)

        # w4: B(p1,h0) streaming, A(p1,h1)
        scale2(0, 1)
        s_unit(1, 1, 0)
        s_unit(1, 1, 1)
        for j in range(16):
            y_unit(1, 0, j)
            if j < 14:
                s_unit(1, 1, j + 2)
        scale1(1, 0)

        # w5: out-proj pair 0 first (wait-free PE), B(p1,h1) streaming
        op_unit(0, 0)
        op_unit(0, 1)
        scale2(1, 0)
        for j in range(16):
            y_unit(1, 1, j)
            if j % 3 == 2:
                op_unit(0, 2 + j // 3)
        op_unit(0, 7)
        scale1(1, 1)
        scale2(1, 1)
        for j in range(8):
            op_unit(1, j, use_act=True)

    nc.compile()
    return nc


_NC = None


def _rope_tables():
    inv = (1.0 / 10000.0) ** (np.arange(0, HD, 2, dtype=np.float64) / HD)
    t = np.arange(T, dtype=np.float64)
    f = np.outer(t, inv)  # (T, 32)
    cc = np.concatenate([np.cos(f), np.cos(f)], axis=1).astype(np.float32)
    sc = np.concatenate([np.sin(f), -np.sin(f)], axis=1).astype(np.float32)
    return cc, sc


def kernel(x, vi, Wq, Wk, Wv, Wo, lamb, sink_weights):
    global _NC
    bf16 = ml_dtypes.bfloat16
    x = np.asarray(x, dtype=np.float32)
    vi = np.asarray(vi, dtype=np.float32)
    Wq = np.asarray(Wq, dtype=np.float32)
    Wk = np.asarray(Wk, dtype=np.float32)
    Wv = np.asarray(Wv, dtype=np.float32)
    Wo = np.asarray(Wo, dtype=np.float32)
    lam = float(np.asarray(lamb).reshape(-1)[0])
    sink = np.asarray(sink_weights, dtype=np.float32).reshape(-1)

    if _NC is None:
        _NC = _build_program()

    x0T = x[0].T  # (D, T)
    xtb = np.ascontiguousarray(
        x0T.reshape(8, 128, NT, 128).transpose(2, 1, 0, 3)
    ).astype(bf16)
    cc, sc = _rope_tables()
    ccb = np.ascontiguousarray(cc.reshape(NT, 128, 64).transpose(1, 0, 2)).astype(bf16)
    scb = np.ascontiguousarray(sc.reshape(NT, 128, 64).transpose(1, 0, 2)).astype(bf16)
    tri = (np.arange(128)[None, :] >= np.arange(128)[:, None]).astype(bf16)
    idn = np.eye(128, dtype=bf16)

    in_maps = []
    for c in range(8):
        lo = 128 * c
        wqkv = np.concatenate(
            [
                Wq[lo : lo + 128].T,
                Wk[lo : lo + 128].T,
                (1.0 - lam) * Wv[lo : lo + 128].T,
            ],
            axis=1,
        )  # (D, 384)
        wqkv = np.ascontiguousarray(
            wqkv.reshape(8, 128, 384).transpose(1, 0, 2)
        ).astype(bf16)
        onp = np.zeros((66, 128), np.float32)
        onp[64, :] = 1.0
        onp[65, 0:64] = np.exp(sink[2 * c])
        onp[65, 64:128] = np.exp(sink[2 * c + 1])
        lsbi = np.ones((1, 4096), np.float32)
        in_maps.append(
            {
                "xtb": xtb,
                "wqkv": wqkv,
                "vis": np.ascontiguousarray(
                    (lam * vi[0][:, lo : lo + 128])
                    .reshape(NT, 128, 128)
                    .transpose(1, 0, 2)
                ).astype(bf16),
                "cc": ccb,
                "sc": scb,
                "wo": np.ascontiguousarray(Wo[:, lo : lo + 128].T).astype(bf16),
                "idn": idn,
                "tri": tri,
                "onp": onp,
                "lsbi": lsbi,
            }
        )

    global _trace_in_maps
    _trace_in_maps = in_maps
    res = None
    for attempt in range(3):
        try:
            res = run_bass_kernel_spmd(_NC, in_maps, list(range(8)))
            break
        except Exception:
            # transient NRT_EXEC_UNIT_UNRECOVERABLE flakes have been seen on
            # the first execute after a fresh compile; retry
            if attempt == 2:
                raise
    outT = np.zeros((D, T), np.float64)
    for c in range(8):
        outT += res.results[c]["out"]
    return np.ascontiguousarray(outT.T).astype(np.float32).reshape(1, T, D)
